# revision 2
# baseline (speedup 1.0000x reference)
"""CFConv (SchNet continuous-filter convolution) on 8 TRN2 NeuronCores.

Reference computation:
    f    = x @ W_in                       # (20000, 128)
    f_j  = f[idx_j]                       # (640000, 128) gather
    wf   = w_ij * f_j                     # elementwise
    conv = segment_sum(wf, seg_i)         # (20000, 128), seg_i sorted
    out  = conv @ W_out + b_out

Distribution: seg_i is sorted, so atoms are split into 8 contiguous
ranges of 2560 (padded to 20480); each core gets the edges targeting its
atom range.  No collectives needed — each core owns its output rows.

Per-core device pipeline (all matmuls bf16, f32 PSUM accumulate):
  Phase A: f = x @ W_in computed locally (replicated), written to an
           internal HBM table (bf16 rows).
  Phase B: edges processed in groups of 128 (one group = one matmul
           contraction).  Groups are host-packed per 128-atom window with
           a fixed per-window group count K_FIX (padding with zero
           edges), so the graph is identical on all cores (SPMD).
    - w_ij group tiles DMA'd from HBM (host-reordered, bf16)
    - f_j rows fetched with gpsimd.dma_gather (MoE gather primitive)
    - wf = w * f_j on VectorE
    - segment-sum via TensorE: psum[fm, atom_window] += wf_g^T @ S_g
      where S_g is the host-built 0/1 edge->atom one-hot matrix
    - out^T = W_out^T @ conv^T (TensorE), bias via ScalarE, transposed
      back per 128x128 tile on TensorE, DMA'd to the output shard.
"""

import numpy as np
import ml_dtypes

import concourse.bacc as bacc
import concourse.bass as bass
import concourse.mybir as mybir
import concourse.tile as tile
from concourse.bass_utils import run_bass_kernel_spmd

BF16 = ml_dtypes.bfloat16

N_ATOMS = 20000
N_EDGES = 640000
F = 128
N_CORES = 8
A_CORE = 2560                 # padded atoms per core
A_PAD = A_CORE * N_CORES      # 20480
CHUNK = 512                   # atoms per PSUM chunk (one bank)
N_CH = A_CORE // CHUNK        # 5
WIN = 128                     # atoms per window (matmul N dim)
WIN_PER_CORE = A_CORE // WIN  # 20
N_WIN = A_PAD // WIN          # 160

TRACE = False                 # set True (with ntff shim) for profiling
_BUILD_CACHE: dict = {}


def _build(k_fix: int):
    """Build the SPMD Bass graph for a given per-window group count."""
    if k_fix in _BUILD_CACHE:
        return _BUILD_CACHE[k_fix]

    G = WIN_PER_CORE * k_fix      # groups per core
    E = G * 128                   # padded edges per core
    bf = mybir.dt.bfloat16
    f32 = mybir.dt.float32

    nc = bacc.Bacc("TRN2", target_bir_lowering=False, debug=False)
    xT_e = nc.dram_tensor("xT", [128, A_PAD], bf, kind="ExternalInput")
    w_in_e = nc.dram_tensor("w_in", [128, 128], bf, kind="ExternalInput")
    w_out_e = nc.dram_tensor("w_out", [128, 128], bf, kind="ExternalInput")
    b_e = nc.dram_tensor("b_out", [128, 1], f32, kind="ExternalInput")
    id_e = nc.dram_tensor("ident", [128, 128], bf, kind="ExternalInput")
    w_ed_e = nc.dram_tensor("w_ed", [128, G, F], bf, kind="ExternalInput")
    s_ed_e = nc.dram_tensor("s_ed", [128, G, WIN], bf, kind="ExternalInput")
    idx_e = nc.dram_tensor("idxw", [128, E // 16], mybir.dt.int16,
                           kind="ExternalInput")
    out_e = nc.dram_tensor("out", [A_CORE, F], f32, kind="ExternalOutput")

    with tile.TileContext(nc) as tc:
        with (
            tc.tile_pool(name="dram", bufs=1, space="DRAM") as dpool,
            tc.tile_pool(name="const", bufs=1) as cpool,
        ):
            f_hbm = dpool.tile([A_PAD, F], bf)

            w_in_t = cpool.tile([128, 128], bf)
            nc.sync.dma_start(w_in_t[:], w_in_e[:])
            w_out_t = cpool.tile([128, 128], bf)
            nc.sync.dma_start(w_out_t[:], w_out_e[:])
            b_t = cpool.tile([128, 1], f32)
            nc.sync.dma_start(b_t[:], b_e[:])
            id_t = cpool.tile([128, 128], bf)
            nc.sync.dma_start(id_t[:], id_e[:])
            idx_t = cpool.tile([128, E // 16], mybir.dt.int16)
            nc.sync.dma_start(idx_t[:], idx_e[:])

            # ---------------- Phase A: f table ----------------
            with (
                tc.tile_pool(name="pha", bufs=2) as apool,
                tc.tile_pool(name="psA", bufs=2, space="PSUM") as psA,
            ):
                xT_t = apool.tile([128, A_PAD], bf)
                nc.sync.dma_start(xT_t[:], xT_e[:])
                n_t = A_PAD // 128  # 160 atom tiles
                for t4 in range(n_t // 4):    # 4 matmuls per psum bank
                    ps = psA.tile([128, 4, 128], f32)
                    for q in range(4):
                        t = t4 * 4 + q
                        nc.tensor.matmul(
                            ps[:, q, :],
                            xT_t[:, t * 128:(t + 1) * 128],
                            w_in_t[:],
                            start=True, stop=True,
                        )
                    j = t4 % 2
                    if j == 0:
                        f_sb = apool.tile([128, 8, F], bf, tag="fsb")
                    nc.vector.tensor_copy(f_sb[:, j * 4:(j + 1) * 4, :], ps[:])
                    if j == 1:
                        a0 = (t4 - 1) * 512
                        dst = f_hbm[a0:a0 + 1024, :].rearrange(
                            "(j p) f -> p j f", p=128)
                        nc.sync.dma_start(dst, f_sb[:])

            # ---------------- Phase B: edges ----------------
            with (
                tc.tile_pool(name="phb", bufs=3) as bpool,
                tc.tile_pool(name="psC", bufs=2, space="PSUM") as pscp,
                tc.tile_pool(name="ps2", bufs=2, space="PSUM") as ps2p,
                tc.tile_pool(name="ps3", bufs=2, space="PSUM") as ps3p,
            ):
                psc = None
                for wk in range(WIN_PER_CORE):
                    ch = wk // 4
                    col = WIN * (wk % 4)

                    w_t = bpool.tile([128, k_fix, F], bf, tag="w")
                    nc.sync.dma_start(
                        w_t[:], w_ed_e[:, wk * k_fix:(wk + 1) * k_fix, :])
                    s_t = bpool.tile([128, k_fix, WIN], bf, tag="s")
                    nc.sync.dma_start(
                        s_t[:], s_ed_e[:, wk * k_fix:(wk + 1) * k_fix, :])
                    fj_t = bpool.tile([128, k_fix, F], bf, tag="fj")
                    nc.gpsimd.dma_gather(
                        fj_t[:], f_hbm[:, :],
                        idx_t[:, wk * k_fix * 8:(wk + 1) * k_fix * 8],
                        num_idxs=k_fix * 128,
                        num_idxs_reg=k_fix * 128,
                        elem_size=F,
                        single_packet=False,
                    )
                    wf_t = bpool.tile([128, k_fix, F], bf, tag="wf")
                    nc.vector.tensor_tensor(
                        wf_t[:], w_t[:], fj_t[:], mybir.AluOpType.mult)

                    if wk % 4 == 0:
                        psc = pscp.tile([128, CHUNK], f32)
                    for g in range(k_fix):
                        nc.tensor.matmul(
                            psc[:, col:col + WIN],
                            wf_t[:, g, :],
                            s_t[:, g, :],
                            start=(g == 0), stop=(g == k_fix - 1),
                        )

                    if wk % 4 == 3:
                        convT = bpool.tile([128, CHUNK], bf, tag="convT")
                        nc.vector.tensor_copy(convT[:], psc[:])
                        ps2 = ps2p.tile([128, CHUNK], f32)
                        nc.tensor.matmul(ps2[:], w_out_t[:], convT[:],
                                         start=True, stop=True)
                        outT = bpool.tile([128, CHUNK], bf, tag="outT")
                        nc.scalar.activation(
                            outT[:], ps2[:],
                            mybir.ActivationFunctionType.Identity,
                            bias=b_t[:],
                        )
                        outf = bpool.tile([128, 4, F], f32, tag="outf")
                        for t in range(4):
                            ps3 = ps3p.tile([128, 128], bf)
                            nc.tensor.transpose(
                                ps3[:], outT[:, t * 128:(t + 1) * 128],
                                id_t[:])
                            nc.vector.tensor_copy(outf[:, t, :], ps3[:])
                        dst = out_e[ch * CHUNK:(ch + 1) * CHUNK, :].rearrange(
                            "(t p) f -> p t f", p=128)
                        nc.sync.dma_start(dst, outf[:])

    nc.compile()
    _BUILD_CACHE[k_fix] = nc
    return nc


def _prep(x, w_ij, seg_i, idx_j, W_in, W_out, b_out):
    """Host-side sharding: reorder/pad edges, build S one-hots, wrap idxs."""
    x = np.asarray(x, dtype=np.float32)
    w_ij = np.asarray(w_ij, dtype=np.float32)
    seg = np.asarray(seg_i).astype(np.int64)
    idxj = np.asarray(idx_j).astype(np.int64)
    if not np.all(np.diff(seg) >= 0):
        order = np.argsort(seg, kind="stable")
        seg, idxj, w_ij = seg[order], idxj[order], w_ij[order]

    bounds = np.searchsorted(seg, np.arange(N_WIN + 1) * WIN)
    cnt = np.diff(bounds)
    k_fix = max(1, int(np.ceil(cnt.max() / 128)))
    e_win = k_fix * 128
    g_core = WIN_PER_CORE * k_fix
    e_pad = g_core * 128

    offs = bounds[:-1, None] + np.arange(e_win)[None, :]      # [160, e_win]
    valid = offs < bounds[1:, None]
    eidx = np.where(valid, offs, 0)

    w_bf = w_ij.astype(BF16)
    seg16 = seg.astype(np.int64)
    idx16 = idxj.astype(np.int16)

    xT = np.zeros((128, A_PAD), BF16)
    xT[:, :N_ATOMS] = np.ascontiguousarray(x.T).astype(BF16)
    shared = {
        "xT": xT,
        "w_in": np.asarray(W_in, np.float32).astype(BF16),
        "w_out": np.asarray(W_out, np.float32).astype(BF16),
        "b_out": np.asarray(b_out, np.float32).reshape(128, 1).copy(),
        "ident": np.eye(128, dtype=BF16),
    }

    in_maps = []
    for c in range(N_CORES):
        sl = slice(c * WIN_PER_CORE, (c + 1) * WIN_PER_CORE)
        ei = eidx[sl].reshape(-1)
        va = valid[sl].reshape(-1)

        w_rows = np.zeros((e_pad, F), BF16)
        w_rows[va] = w_bf[ei[va]]
        w_ed = np.ascontiguousarray(
            w_rows.reshape(g_core, 128, F).transpose(1, 0, 2))

        wb = (np.arange(c * WIN_PER_CORE, (c + 1) * WIN_PER_CORE)
              * WIN).repeat(e_win)
        rel = (seg16[ei] - wb)
        s_rows = np.zeros((e_pad, WIN), BF16)
        vrows = np.nonzero(va)[0]
        s_rows[vrows, rel[vrows]] = 1
        s_ed = np.ascontiguousarray(
            s_rows.reshape(g_core, 128, WIN).transpose(1, 0, 2))

        ii = np.zeros(e_pad, np.int16)
        ii[va] = idx16[ei[va]]
        idxw = np.ascontiguousarray(
            np.tile(ii.reshape(-1, 16).T, (8, 1)))

        m = dict(shared)
        m["w_ed"] = w_ed
        m["s_ed"] = s_ed
        m["idxw"] = idxw
        in_maps.append(m)
    return k_fix, in_maps


def kernel(x, w_ij, seg_i, idx_j, seg_i_sum, W_in, W_out, b_out):
    k_fix, in_maps = _prep(x, w_ij, seg_i, idx_j, W_in, W_out, b_out)
    nc = _build(k_fix)
    res = run_bass_kernel_spmd(nc, in_maps, core_ids=list(range(N_CORES)),
                               trace=TRACE)
    kernel.last_result = res
    out = np.concatenate(
        [np.asarray(res.results[c]["out"]) for c in range(N_CORES)], axis=0)
    return np.ascontiguousarray(out[:N_ATOMS]).astype(np.float32)


# revision 5
# speedup vs baseline: 2.1170x; 2.1170x over previous
"""CFConv (SchNet continuous-filter convolution) on 8 TRN2 NeuronCores.

Reference computation:
    f    = x @ W_in                       # (20000, 128)
    f_j  = f[idx_j]                       # (640000, 128) gather
    wf   = w_ij * f_j                     # elementwise
    conv = segment_sum(wf, seg_i)         # (20000, 128), seg_i sorted
    out  = conv @ W_out + b_out

Distribution: seg_i is sorted, so atoms are split into 8 contiguous
ranges of 2560 (padded to 20480); each core gets the edges targeting its
atom range.  No collectives needed — each core owns its output rows.

Per-core device pipeline (all matmuls bf16, f32 PSUM accumulate):
  Phase A: f = x @ W_in computed locally (replicated), written to an
           internal HBM table (bf16 rows).
  Phase B: edges processed in groups of 128 (one group = one matmul
           contraction).  Groups are host-packed per 128-atom window with
           a fixed per-window group count K_FIX (padding with zero
           edges), so the graph is identical on all cores (SPMD).
    - w_ij group tiles DMA'd from HBM (host-reordered, bf16)
    - f_j rows fetched with gpsimd.dma_gather (MoE gather primitive)
    - wf = w * f_j on VectorE
    - segment-sum via TensorE: psum[fm, atom_window] += wf_g^T @ S_g
      where S_g is the host-built 0/1 edge->atom one-hot matrix
    - out^T = W_out^T @ conv^T (TensorE), bias via ScalarE, transposed
      back per 128x128 tile on TensorE, DMA'd to the output shard.
"""

import numpy as np
import ml_dtypes

import concourse.bacc as bacc
import concourse.bass as bass
import concourse.mybir as mybir
import concourse.tile as tile
from concourse.bass_utils import run_bass_kernel_spmd

BF16 = ml_dtypes.bfloat16

N_ATOMS = 20000
N_EDGES = 640000
F = 128
N_CORES = 8
A_CORE = 2560                 # padded atoms per core
A_PAD = A_CORE * N_CORES      # 20480
CHUNK = 512                   # atoms per PSUM chunk (one bank)
N_CH = A_CORE // CHUNK        # 5
WIN = 128                     # atoms per window (matmul N dim)
WIN_PER_CORE = A_CORE // WIN  # 20
N_WIN = A_PAD // WIN          # 160

TRACE = False                 # set True (with ntff shim) for profiling
_BUILD_CACHE: dict = {}


def _build(k_fix: int):
    """Build the SPMD Bass graph for a given per-window group count."""
    if k_fix in _BUILD_CACHE:
        return _BUILD_CACHE[k_fix]

    G = WIN_PER_CORE * k_fix      # groups per core
    E = G * 128                   # padded edges per core
    bf = mybir.dt.bfloat16
    f32 = mybir.dt.float32

    nc = bacc.Bacc("TRN2", target_bir_lowering=False, debug=False,
                   num_swdge_queues=4)
    xT_e = nc.dram_tensor("xT", [128, A_PAD], bf, kind="ExternalInput")
    w_in_e = nc.dram_tensor("w_in", [128, 128], bf, kind="ExternalInput")
    w_out_e = nc.dram_tensor("w_out", [128, 128], bf, kind="ExternalInput")
    b_e = nc.dram_tensor("b_out", [128, 1], f32, kind="ExternalInput")
    id_e = nc.dram_tensor("ident", [128, 128], bf, kind="ExternalInput")
    w_ed_e = nc.dram_tensor("w_ed", [128, G, F], bf, kind="ExternalInput")
    s_ed_e = nc.dram_tensor("s_ed", [128, G, WIN], bf, kind="ExternalInput")
    idx_e = nc.dram_tensor("idxw", [128, E // 16], mybir.dt.int16,
                           kind="ExternalInput")
    out_e = nc.dram_tensor("out", [A_CORE, F], f32, kind="ExternalOutput")

    with tile.TileContext(nc) as tc:
        with (
            tc.tile_pool(name="dram", bufs=1, space="DRAM") as dpool,
            tc.tile_pool(name="const", bufs=1) as cpool,
        ):
            f_hbm = dpool.tile([A_PAD, F], bf)

            w_in_t = cpool.tile([128, 128], bf)
            nc.sync.dma_start(w_in_t[:], w_in_e[:])
            w_out_t = cpool.tile([128, 128], bf)
            nc.sync.dma_start(w_out_t[:], w_out_e[:])
            b_t = cpool.tile([128, 1], f32)
            nc.sync.dma_start(b_t[:], b_e[:])
            id_t = cpool.tile([128, 128], bf)
            nc.sync.dma_start(id_t[:], id_e[:])
            idx_t = cpool.tile([128, E // 16], mybir.dt.int16)
            nc.sync.dma_start(idx_t[:], idx_e[:])

            # ---------------- Phase A: f table ----------------
            with (
                tc.tile_pool(name="pha", bufs=2) as apool,
                tc.tile_pool(name="psA", bufs=2, space="PSUM") as psA,
            ):
                xT_t = apool.tile([128, A_PAD], bf)
                nc.sync.dma_start(xT_t[:], xT_e[:])
                n_t = A_PAD // 128  # 160 atom tiles
                for t4 in range(n_t // 4):    # 4 matmuls per psum bank
                    ps = psA.tile([128, 4, 128], f32)
                    for q in range(4):
                        t = t4 * 4 + q
                        nc.tensor.matmul(
                            ps[:, q, :],
                            xT_t[:, t * 128:(t + 1) * 128],
                            w_in_t[:],
                            start=True, stop=True,
                        )
                    j = t4 % 2
                    if j == 0:
                        f_sb = apool.tile([128, 8, F], bf, tag="fsb")
                    nc.vector.tensor_copy(f_sb[:, j * 4:(j + 1) * 4, :], ps[:])
                    if j == 1:
                        a0 = (t4 - 1) * 512
                        dst = f_hbm[a0:a0 + 1024, :].rearrange(
                            "(j p) f -> p j f", p=128)
                        nc.sync.dma_start(dst, f_sb[:])

            # ---------------- Phase B: edges ----------------
            with (
                tc.tile_pool(name="phb", bufs=3) as bpool,
                tc.tile_pool(name="fjp", bufs=6) as fjpool,
                tc.tile_pool(name="psC", bufs=2, space="PSUM") as pscp,
                tc.tile_pool(name="ps2", bufs=2, space="PSUM") as ps2p,
                tc.tile_pool(name="ps3", bufs=2, space="PSUM") as ps3p,
            ):
                psc = None
                for wk in range(WIN_PER_CORE):
                    ch = wk // 4
                    col = WIN * (wk % 4)

                    w_t = bpool.tile([128, k_fix, F], bf, tag="w")
                    nc.sync.dma_start(
                        w_t[:], w_ed_e[:, wk * k_fix:(wk + 1) * k_fix, :])
                    s_t = bpool.tile([128, k_fix, WIN], bf, tag="s")
                    nc.sync.dma_start(
                        s_t[:], s_ed_e[:, wk * k_fix:(wk + 1) * k_fix, :])
                    fj_t = fjpool.tile([128, k_fix, F], bf, tag="fj")
                    nc.gpsimd.dma_gather(
                        fj_t[:], f_hbm[:, :],
                        idx_t[:, wk * k_fix * 8:(wk + 1) * k_fix * 8],
                        num_idxs=k_fix * 128,
                        num_idxs_reg=k_fix * 128,
                        elem_size=F,
                        single_packet=False,
                        queue_num=wk % 4,
                    )
                    wf_t = bpool.tile([128, k_fix, F], bf, tag="wf")
                    nc.vector.tensor_tensor(
                        wf_t[:], w_t[:], fj_t[:], mybir.AluOpType.mult)

                    if wk % 4 == 0:
                        psc = pscp.tile([128, CHUNK], f32)
                    for g in range(k_fix):
                        nc.tensor.matmul(
                            psc[:, col:col + WIN],
                            wf_t[:, g, :],
                            s_t[:, g, :],
                            start=(g == 0), stop=(g == k_fix - 1),
                        )

                    if wk % 4 == 3:
                        convT = bpool.tile([128, CHUNK], bf, tag="convT")
                        nc.vector.tensor_copy(convT[:], psc[:])
                        ps2 = ps2p.tile([128, CHUNK], f32)
                        nc.tensor.matmul(ps2[:], w_out_t[:], convT[:],
                                         start=True, stop=True)
                        outT = bpool.tile([128, CHUNK], bf, tag="outT")
                        nc.scalar.activation(
                            outT[:], ps2[:],
                            mybir.ActivationFunctionType.Identity,
                            bias=b_t[:],
                        )
                        outf = bpool.tile([128, 4, F], f32, tag="outf")
                        for t in range(4):
                            ps3 = ps3p.tile([128, 128], bf)
                            nc.tensor.transpose(
                                ps3[:], outT[:, t * 128:(t + 1) * 128],
                                id_t[:])
                            nc.vector.tensor_copy(outf[:, t, :], ps3[:])
                        dst = out_e[ch * CHUNK:(ch + 1) * CHUNK, :].rearrange(
                            "(t p) f -> p t f", p=128)
                        nc.sync.dma_start(dst, outf[:])

    nc.compile()
    _BUILD_CACHE[k_fix] = nc
    return nc


def _prep(x, w_ij, seg_i, idx_j, W_in, W_out, b_out):
    """Host-side sharding: reorder/pad edges, build S one-hots, wrap idxs."""
    x = np.asarray(x, dtype=np.float32)
    w_ij = np.asarray(w_ij, dtype=np.float32)
    seg = np.asarray(seg_i).astype(np.int64)
    idxj = np.asarray(idx_j).astype(np.int64)
    if not np.all(np.diff(seg) >= 0):
        order = np.argsort(seg, kind="stable")
        seg, idxj, w_ij = seg[order], idxj[order], w_ij[order]

    bounds = np.searchsorted(seg, np.arange(N_WIN + 1) * WIN)
    cnt = np.diff(bounds)
    k_fix = max(1, int(np.ceil(cnt.max() / 128)))
    e_win = k_fix * 128
    g_core = WIN_PER_CORE * k_fix
    e_pad = g_core * 128

    offs = bounds[:-1, None] + np.arange(e_win)[None, :]      # [160, e_win]
    valid = offs < bounds[1:, None]
    eidx = np.where(valid, offs, 0)

    w_bf = w_ij.astype(BF16)
    seg16 = seg.astype(np.int64)
    idx16 = idxj.astype(np.int16)

    xT = np.zeros((128, A_PAD), BF16)
    xT[:, :N_ATOMS] = np.ascontiguousarray(x.T).astype(BF16)
    shared = {
        "xT": xT,
        "w_in": np.asarray(W_in, np.float32).astype(BF16),
        "w_out": np.asarray(W_out, np.float32).astype(BF16),
        "b_out": np.asarray(b_out, np.float32).reshape(128, 1).copy(),
        "ident": np.eye(128, dtype=BF16),
    }

    in_maps = []
    for c in range(N_CORES):
        sl = slice(c * WIN_PER_CORE, (c + 1) * WIN_PER_CORE)
        ei = eidx[sl].reshape(-1)
        va = valid[sl].reshape(-1)

        w_rows = np.zeros((e_pad, F), BF16)
        w_rows[va] = w_bf[ei[va]]
        w_ed = np.ascontiguousarray(
            w_rows.reshape(g_core, 128, F).transpose(1, 0, 2))

        wb = (np.arange(c * WIN_PER_CORE, (c + 1) * WIN_PER_CORE)
              * WIN).repeat(e_win)
        rel = (seg16[ei] - wb)
        s_rows = np.zeros((e_pad, WIN), BF16)
        vrows = np.nonzero(va)[0]
        s_rows[vrows, rel[vrows]] = 1
        s_ed = np.ascontiguousarray(
            s_rows.reshape(g_core, 128, WIN).transpose(1, 0, 2))

        ii = np.zeros(e_pad, np.int16)
        ii[va] = idx16[ei[va]]
        idxw = np.ascontiguousarray(
            np.tile(ii.reshape(-1, 16).T, (8, 1)))

        m = dict(shared)
        m["w_ed"] = w_ed
        m["s_ed"] = s_ed
        m["idxw"] = idxw
        in_maps.append(m)
    return k_fix, in_maps


def kernel(x, w_ij, seg_i, idx_j, seg_i_sum, W_in, W_out, b_out):
    k_fix, in_maps = _prep(x, w_ij, seg_i, idx_j, W_in, W_out, b_out)
    nc = _build(k_fix)
    res = run_bass_kernel_spmd(nc, in_maps, core_ids=list(range(N_CORES)),
                               trace=TRACE)
    kernel.last_result = res
    out = np.concatenate(
        [np.asarray(res.results[c]["out"]) for c in range(N_CORES)], axis=0)
    return np.ascontiguousarray(out[:N_ATOMS]).astype(np.float32)


# revision 11
# speedup vs baseline: 2.3105x; 1.0914x over previous
"""CFConv (SchNet continuous-filter convolution) on 8 TRN2 NeuronCores.

Reference computation:
    f    = x @ W_in                       # (20000, 128)
    f_j  = f[idx_j]                       # (640000, 128) gather
    wf   = w_ij * f_j                     # elementwise
    conv = segment_sum(wf, seg_i)         # (20000, 128), seg_i sorted
    out  = conv @ W_out + b_out

Distribution: seg_i is sorted, so atoms are split into 8 contiguous
ranges of 2560 (padded to 20480); each core gets the edges targeting its
atom range.  No collectives needed — each core owns its output rows.

Per-core device pipeline (all matmuls bf16, f32 PSUM accumulate):
  Phase A: f = x @ W_in computed locally (replicated), written to an
           internal HBM table (bf16 rows).
  Phase B: edges processed in groups of 128 (one group = one matmul
           contraction).  Groups are host-packed per 128-atom window with
           a fixed per-window group count K_FIX (padding with zero
           edges), so the graph is identical on all cores (SPMD).
    - w_ij group tiles DMA'd from HBM (host-reordered, bf16)
    - f_j rows fetched with gpsimd.dma_gather (MoE gather primitive)
    - wf = w * f_j on VectorE
    - segment-sum via TensorE: psum[fm, atom_window] += wf_g^T @ S_g
      where S_g is the host-built 0/1 edge->atom one-hot matrix
    - out^T = W_out^T @ conv^T (TensorE), bias via ScalarE, transposed
      back per 128x128 tile on TensorE, DMA'd to the output shard.
"""

import numpy as np
import ml_dtypes

import concourse.bacc as bacc
import concourse.bass as bass
import concourse.mybir as mybir
import concourse.tile as tile
from concourse.bass_utils import run_bass_kernel_spmd

BF16 = ml_dtypes.bfloat16

N_ATOMS = 20000
N_EDGES = 640000
F = 128
N_CORES = 8
A_CORE = 2560                 # padded atoms per core
A_PAD = A_CORE * N_CORES      # 20480
CHUNK = 512                   # atoms per PSUM chunk (one bank)
N_CH = A_CORE // CHUNK        # 5
WIN = 128                     # atoms per window (matmul N dim)
WIN_PER_CORE = A_CORE // WIN  # 20
N_WIN = A_PAD // WIN          # 160

TRACE = False                 # set True (with ntff shim) for profiling
_BUILD_CACHE: dict = {}


def _build(k_fix: int):
    """Build the SPMD Bass graph for a given per-window group count."""
    if k_fix in _BUILD_CACHE:
        return _BUILD_CACHE[k_fix]

    G = WIN_PER_CORE * k_fix      # groups per core
    E = G * 128                   # padded edges per core
    bf = mybir.dt.bfloat16
    f32 = mybir.dt.float32

    nc = bacc.Bacc("TRN2", target_bir_lowering=False, debug=False,
                   num_swdge_queues=4)
    xT_e = nc.dram_tensor("xT", [128, A_PAD], bf, kind="ExternalInput")
    w_in_e = nc.dram_tensor("w_in", [128, 128], bf, kind="ExternalInput")
    w_out_e = nc.dram_tensor("w_out", [128, 128], bf, kind="ExternalInput")
    b_e = nc.dram_tensor("b_out", [128, 1], f32, kind="ExternalInput")
    id_e = nc.dram_tensor("ident", [128, 128], bf, kind="ExternalInput")
    w_ed_e = nc.dram_tensor("w_ed", [128, G, F], bf, kind="ExternalInput")
    rel_e = nc.dram_tensor("rel", [128, G], bf, kind="ExternalInput")
    iota_e = nc.dram_tensor("iotat", [128, k_fix, WIN], bf,
                            kind="ExternalInput")
    idx_e = nc.dram_tensor("idxw", [128, E // 16], mybir.dt.int16,
                           kind="ExternalInput")
    out_e = nc.dram_tensor("out", [A_CORE, F], f32, kind="ExternalOutput")

    with tile.TileContext(nc) as tc:
        with (
            tc.tile_pool(name="dram", bufs=1, space="DRAM") as dpool,
            tc.tile_pool(name="const", bufs=1) as cpool,
        ):
            f_hbm = dpool.tile([A_PAD, F], bf)

            w_in_t = cpool.tile([128, 128], bf)
            nc.sync.dma_start(w_in_t[:], w_in_e[:])
            w_out_t = cpool.tile([128, 128], bf)
            nc.sync.dma_start(w_out_t[:], w_out_e[:])
            b_t = cpool.tile([128, 1], f32)
            nc.sync.dma_start(b_t[:], b_e[:])
            id_t = cpool.tile([128, 128], bf)
            nc.sync.dma_start(id_t[:], id_e[:])
            idx_t = cpool.tile([128, E // 16], mybir.dt.int16)
            nc.sync.dma_start(idx_t[:], idx_e[:])
            rel_t = cpool.tile([128, G], bf)
            nc.sync.dma_start(rel_t[:], rel_e[:])
            iota_t = cpool.tile([128, k_fix, WIN], bf)
            nc.sync.dma_start(iota_t[:], iota_e[:])

            # ---------------- Phase A: f table ----------------
            with (
                tc.tile_pool(name="pha", bufs=2) as apool,
                tc.tile_pool(name="psA", bufs=2, space="PSUM") as psA,
            ):
                xT_t = apool.tile([128, A_PAD], bf)
                nc.sync.dma_start(xT_t[:], xT_e[:])
                n_t = A_PAD // 128  # 160 atom tiles
                for t4 in range(n_t // 4):    # 4 matmuls per psum bank
                    ps = psA.tile([128, 4, 128], f32)
                    for q in range(4):
                        t = t4 * 4 + q
                        nc.tensor.matmul(
                            ps[:, q, :],
                            xT_t[:, t * 128:(t + 1) * 128],
                            w_in_t[:],
                            start=True, stop=True,
                        )
                    j = t4 % 2
                    if j == 0:
                        f_sb = apool.tile([128, 8, F], bf, tag="fsb")
                    nc.vector.tensor_copy(f_sb[:, j * 4:(j + 1) * 4, :], ps[:])
                    if j == 1:
                        a0 = (t4 - 1) * 512
                        dst = f_hbm[a0:a0 + 1024, :].rearrange(
                            "(j p) f -> p j f", p=128)
                        nc.sync.dma_start(dst, f_sb[:])

            # ---------------- Phase B: edges ----------------
            with (
                tc.tile_pool(name="phb", bufs=3) as bpool,
                tc.tile_pool(name="fjp", bufs=6) as fjpool,
                tc.tile_pool(name="psC", bufs=2, space="PSUM") as pscp,
                tc.tile_pool(name="ps2", bufs=2, space="PSUM") as ps2p,
                tc.tile_pool(name="ps3", bufs=2, space="PSUM") as ps3p,
            ):
                psc = None
                for wk in range(WIN_PER_CORE):
                    ch = wk // 4
                    col = WIN * (wk % 4)

                    w_t = bpool.tile([128, k_fix, F], bf, tag="w")
                    nc.sync.dma_start(
                        w_t[:], w_ed_e[:, wk * k_fix:(wk + 1) * k_fix, :])
                    s_t = bpool.tile([128, k_fix, WIN], bf, tag="s")
                    nc.vector.tensor_tensor(
                        s_t[:],
                        rel_t[:, wk * k_fix:(wk + 1) * k_fix].to_broadcast(
                            [128, k_fix, WIN]),
                        iota_t[:],
                        mybir.AluOpType.is_equal)
                    fj_t = fjpool.tile([128, k_fix, F], bf, tag="fj")
                    nc.gpsimd.dma_gather(
                        fj_t[:], f_hbm[:, :],
                        idx_t[:, wk * k_fix * 8:(wk + 1) * k_fix * 8],
                        num_idxs=k_fix * 128,
                        num_idxs_reg=k_fix * 128,
                        elem_size=F,
                        single_packet=False,
                        queue_num=wk % 4,
                    )
                    wf_t = bpool.tile([128, k_fix, F], bf, tag="wf")
                    nc.vector.tensor_tensor(
                        wf_t[:], w_t[:], fj_t[:], mybir.AluOpType.mult)

                    if wk % 4 == 0:
                        psc = pscp.tile([128, CHUNK], f32)
                    for g in range(k_fix):
                        nc.tensor.matmul(
                            psc[:, col:col + WIN],
                            wf_t[:, g, :],
                            s_t[:, g, :],
                            start=(g == 0), stop=(g == k_fix - 1),
                        )

                    if wk % 4 == 3:
                        convT = bpool.tile([128, CHUNK], bf, tag="convT")
                        nc.vector.tensor_copy(convT[:], psc[:])
                        ps2 = ps2p.tile([128, CHUNK], f32)
                        nc.tensor.matmul(ps2[:], w_out_t[:], convT[:],
                                         start=True, stop=True)
                        outT = bpool.tile([128, CHUNK], bf, tag="outT")
                        nc.scalar.activation(
                            outT[:], ps2[:],
                            mybir.ActivationFunctionType.Identity,
                            bias=b_t[:],
                        )
                        outf = bpool.tile([128, 4, F], f32, tag="outf")
                        for t in range(4):
                            ps3 = ps3p.tile([128, 128], bf)
                            nc.tensor.transpose(
                                ps3[:], outT[:, t * 128:(t + 1) * 128],
                                id_t[:])
                            nc.vector.tensor_copy(outf[:, t, :], ps3[:])
                        dst = out_e[ch * CHUNK:(ch + 1) * CHUNK, :].rearrange(
                            "(t p) f -> p t f", p=128)
                        nc.sync.dma_start(dst, outf[:])

    nc.compile()
    _BUILD_CACHE[k_fix] = nc
    return nc


def _prep(x, w_ij, seg_i, idx_j, W_in, W_out, b_out):
    """Host-side sharding: reorder/pad edges, build S one-hots, wrap idxs."""
    x = np.asarray(x, dtype=np.float32)
    w_ij = np.asarray(w_ij, dtype=np.float32)
    seg = np.asarray(seg_i).astype(np.int64)
    idxj = np.asarray(idx_j).astype(np.int64)
    if not np.all(np.diff(seg) >= 0):
        order = np.argsort(seg, kind="stable")
        seg, idxj, w_ij = seg[order], idxj[order], w_ij[order]

    bounds = np.searchsorted(seg, np.arange(N_WIN + 1) * WIN)
    cnt = np.diff(bounds)
    k_fix = max(1, int(np.ceil(cnt.max() / 128)))
    e_win = k_fix * 128
    g_core = WIN_PER_CORE * k_fix
    e_pad = g_core * 128

    offs = bounds[:-1, None] + np.arange(e_win)[None, :]      # [160, e_win]
    valid = offs < bounds[1:, None]
    eidx = np.where(valid, offs, 0)

    w_bf = w_ij.astype(BF16)
    seg16 = seg.astype(np.int64)
    idx16 = idxj.astype(np.int16)

    xT = np.zeros((128, A_PAD), BF16)
    xT[:, :N_ATOMS] = np.ascontiguousarray(x.T).astype(BF16)
    shared = {
        "xT": xT,
        "w_in": np.asarray(W_in, np.float32).astype(BF16),
        "w_out": np.asarray(W_out, np.float32).astype(BF16),
        "b_out": np.asarray(b_out, np.float32).reshape(128, 1).copy(),
        "ident": np.eye(128, dtype=BF16),
        "iotat": np.ascontiguousarray(
            np.broadcast_to(np.arange(WIN, dtype=np.float32).astype(BF16),
                            (128, k_fix, WIN))),
    }

    in_maps = []
    for c in range(N_CORES):
        sl = slice(c * WIN_PER_CORE, (c + 1) * WIN_PER_CORE)
        ei = eidx[sl].reshape(-1)
        va = valid[sl].reshape(-1)

        w_rows = np.zeros((e_pad, F), BF16)
        w_rows[va] = w_bf[ei[va]]
        w_ed = np.ascontiguousarray(
            w_rows.reshape(g_core, 128, F).transpose(1, 0, 2))

        wb = (np.arange(c * WIN_PER_CORE, (c + 1) * WIN_PER_CORE)
              * WIN).repeat(e_win)
        rel = np.where(va, seg16[ei] - wb, 0)
        rel_ed = np.ascontiguousarray(
            rel.reshape(g_core, 128).T).astype(BF16)

        ii = np.zeros(e_pad, np.int16)
        ii[va] = idx16[ei[va]]
        idxw = np.ascontiguousarray(
            np.tile(ii.reshape(-1, 16).T, (8, 1)))

        m = dict(shared)
        m["w_ed"] = w_ed
        m["rel"] = rel_ed
        m["idxw"] = idxw
        in_maps.append(m)
    return k_fix, in_maps


def kernel(x, w_ij, seg_i, idx_j, seg_i_sum, W_in, W_out, b_out):
    k_fix, in_maps = _prep(x, w_ij, seg_i, idx_j, W_in, W_out, b_out)
    nc = _build(k_fix)
    res = run_bass_kernel_spmd(nc, in_maps, core_ids=list(range(N_CORES)),
                               trace=TRACE)
    kernel.last_result = res
    out = np.concatenate(
        [np.asarray(res.results[c]["out"]) for c in range(N_CORES)], axis=0)
    return np.ascontiguousarray(out[:N_ATOMS]).astype(np.float32)
